# revision 26
# baseline (speedup 1.0000x reference)
"""Trainium2 Bass kernel for nn_AttentionCrossLayer.

Math: in the reference, softmax over a length-1 axis is exactly 1.0, so
attn == v and q/k/wq/wk are dead code. With x0 the (never-mutated) input,
each layer's gate xw_i = out_i @ cw_i is a fixed linear function of x0:
    xw_i = x0 @ u_i + c_i,   u_i = Wv_i @ (Wo_i @ cw_i),
                             c_i = (bv_i @ Wo_i + bo_i) @ cw_i
and the layer recurrence x += x0 * xw_i + cb_i telescopes to
    out[b, d] = x0[b, d] * (x0[b, :] @ usum + cprime) + cbsum[d]
with usum = sum_i u_i  [D], cprime = 1 + sum_i c_i, cbsum = sum_i cb_i [D].

The tiny weight contractions happen host-side in float64. The device
kernel is one pass over x per core, 32 row-tiles of [128, 1024]:
  pass 1 (Vector): fused multiply + row-reduce (scalar_tensor_tensor
    with accum_out) -> per-row gate t. cprime rides in a constant
    column appended to x/u so the reduce emits the finished gate.
  pass 2: in-place x <- x * t + cbsum. When cbsum == 0 (the spec fills
    cb with zeros) this is a per-row scale on the Scalar engine
    (activation per-partition scale AP); a Vector path handles
    cbsum != 0.

Schedule (evidence from per-queue trace A/B over five variants):
  - Two HWDGE load rings (sync+scalar) together sustain the ~420 GB/s
    HBM rate from the first microseconds; SWDGE loads concurrent with
    any other ring degrade the pool to ~300-345 GB/s, so gpsimd never
    loads. SWDGE stores sustain 420 GB/s alone or mixed.
  - An HWDGE ring BLOCKS its issuing engine once the ring backs up, so
    scalar interleaves its load issues with the pass-2 muls (a blocked
    issue loop would starve cm2 and with it the store stream).
  - u arrives host-replicated to [128, D] and streams in as sync's
    first (contiguous) load — no HBM-latency-bound replication
    descriptors polluting the load stream (that cost the old version
    ~15us of ramp), no slow fp32 PE path.
  - One semaphore per store: the DGE serializes DMAs that share a
    semaphore (~2.5us/DMA), which caps a shared-sem store stream at
    ~200 GB/s. Final wait is a chain of per-store waits.
  - Block(no_gpsimd_drain=True): skips a ~3.6us GpSimd dge_drain in
    the epilogue; the store-sem waits already guarantee completion.

Sharding: data-parallel over batch across 8 cores, weights replicated,
no cross-device comms.
"""

import numpy as np

L, B, D, H, K = 3, 32768, 1024, 8, 64
N_CORES = 8
B_LOC = B // N_CORES  # 4096 rows per core
P = 128
N_TILES = B_LOC // P  # 32
DP = D + 32  # slot stride 4224B = 128B aligned; col D holds the 1.0 constant

_cache = {}


def _build_program(cprime: float, zero_cb: bool):
    import concourse.bass as bass
    from concourse import mybir

    F32 = mybir.dt.float32
    BF16 = mybir.dt.bfloat16
    MUL = mybir.AluOpType.mult
    ADD = mybir.AluOpType.add

    nc = bass.Bass()
    x = nc.declare_dram_parameter("x", [B_LOC, D], F32, isOutput=False)
    # u arrives host-replicated to [P, D+1] bf16 (262KB) with the
    # cprime column baked in: one plain contiguous streaming load —
    # no HBM-latency-bound replication descriptors (those poisoned the
    # whole DMA pool for ~15us) and no slow fp32 PE broadcast. bf16 is
    # fine: |u| errors ~0.4% against a 2e-2 relative budget, and the
    # DVE converts mixed-dtype STT operands on read.
    u = nc.declare_dram_parameter("u", [P, D + 1], BF16, isOutput=False)
    cb = nc.declare_dram_parameter("cb", [1, D], F32, isOutput=False)
    out = nc.declare_dram_parameter("out", [B_LOC, D], F32, isOutput=True)

    cb_bcast = bass.AP(tensor=cb.ap().tensor, offset=0, ap=[[0, P], [1, D]])

    # sync: evens plus 31; scalar: odds 1..29. Balanced so both load
    # rings drain their queues at ~the same time (scalar's starts a
    # beat later and pays a small solo-rate tail), since the
    # last-arriving tile gates the pass1->pass2->store chain tail.
    sy_tiles = [i for i in range(N_TILES) if i % 2 == 0] + [31]
    sy_tiles.sort()
    sc_tiles = [i for i in range(1, 30) if i % 2 == 1]  # scalar: odds 1..29

    with (
        nc.sbuf_tensor([P, D + 1], BF16) as ub,  # [:, :D]=usum, [:, D]=cprime
        nc.sbuf_tensor([P, D], F32) as cbb,
        nc.sbuf_tensor([P, N_TILES, DP], F32) as xt,  # [:, i, D] = 1.0
        nc.sbuf_tensor([P, 2, D + 1], F32) as oscr,  # throwaway STT main out
        nc.sbuf_tensor([P, N_TILES, 1], F32) as tsc,
        nc.semaphore("ubb") as ubb,  # u block landed (sync ring)
        nc.semaphore("cbs") as cbs,  # cb broadcast landed (general path)
        nc.semaphore("cm") as cm,  # pass-1 reduces retired (Vector)
        nc.semaphore("cm2") as cm2,  # pass-2 writes retired
        nc.Block(no_gpsimd_drain=True) as block,
    ):
        lds = [nc.alloc_semaphore(f"ld{i}") for i in range(N_TILES)]
        sts = [nc.alloc_semaphore(f"st{i}") for i in range(N_TILES)]

        @block.sync
        def _(sync):
            # u block first: lands by ~9.5us, gating only pass 1.
            sync.dma_start(out=ub[:, :], in_=u.ap()).then_inc(ubb, 16)
            for i in sy_tiles:
                sync.dma_start(
                    out=xt[:, i, 0:D], in_=x[i * P : (i + 1) * P, :]
                ).then_inc(lds[i], 16)

        @block.scalar
        def _(scalar):
            if not zero_cb:
                scalar.dma_start(out=cbb[:, :], in_=cb_bcast).then_inc(cbs, 16)
            # tiles 1,3 up front (ring is empty, issues don't block);
            # the rest interleave with the pass-2 muls below so a
            # backed-up ring can never starve cm2.
            head, rest = sc_tiles[:2], sc_tiles[2:]
            for i in head:
                scalar.dma_start(
                    out=xt[:, i, 0:D], in_=x[i * P : (i + 1) * P, :]
                ).then_inc(lds[i], 16)
            if zero_cb:
                for i in range(N_TILES):
                    if i < len(rest):
                        j = rest[i]
                        scalar.dma_start(
                            out=xt[:, j, 0:D], in_=x[j * P : (j + 1) * P, :]
                        ).then_inc(lds[j], 16)
                    scalar.wait_ge(cm, i + 1)
                    # pass 2: x <- x * t (cbsum == 0)
                    nc.scalar.mul(
                        out=xt[:, i, 0:D],
                        in_=xt[:, i, 0:D],
                        mul=tsc[:, i, :],
                    ).then_inc(cm2, 1)
            else:
                for j in rest:
                    scalar.dma_start(
                        out=xt[:, j, 0:D], in_=x[j * P : (j + 1) * P, :]
                    ).then_inc(lds[j], 16)

        @block.vector
        def _(vector):
            nc.vector.memset(xt[:, :, D : D + 1], 1.0)
            vector.wait_ge(ubb, 16)
            if not zero_cb:
                vector.wait_ge(cbs, 16)
            for i in range(N_TILES):
                vector.wait_ge(lds[i], 16)
                # oscr = x' * u' ; t_i = sum_free = x.usum + cprime
                nc.vector.scalar_tensor_tensor(
                    out=oscr[:, i % 2, :],
                    in0=xt[:, i, 0 : D + 1],
                    scalar=1.0,
                    in1=ub[:, :],
                    op0=MUL,
                    op1=MUL,
                    accum_out=tsc[:, i, :],
                ).then_inc(cm, 1)
                if not zero_cb:
                    # accumulator writeback must retire before t is read
                    vector.wait_ge(cm, i + 1)
                    # in place: x <- x * t + cbsum
                    nc.vector.scalar_tensor_tensor(
                        out=xt[:, i, 0:D],
                        in0=xt[:, i, 0:D],
                        scalar=tsc[:, i, :],
                        in1=cbb[:, :],
                        op0=MUL,
                        op1=ADD,
                    ).then_inc(cm2, 1)

        @block.gpsimd
        def _(gpsimd):
            # Flow control: keep the store stream >= 8 tiles behind the
            # load front. The SDMA pool round-robins between rings with
            # queued work, so a store ring that races the compute chain
            # steals service from the load rings and stretches the
            # load phase (the critical path); trailing stores instead
            # drain opportunistically and finish under the epilogue.
            for i in range(N_TILES):
                gpsimd.wait_ge(lds[min(i + 8, N_TILES - 1)], 16)
                gpsimd.wait_ge(cm2, i + 1)
                gpsimd.dma_start(
                    out=out[i * P : (i + 1) * P, :], in_=xt[:, i, 0:D]
                ).then_inc(sts[i], 16)
            # No final store-completion waits: the program epilogue (NRT
            # postamble) runs concurrently with the trailing stores, and
            # the runtime quiesces DMA before execution completes, so
            # the output is fully in HBM before the host can read it.

    return nc


def _precompute(wv, bv, wo, bo, cw, cb):
    """Host-side f64 contraction of the small per-layer weights."""
    usum = np.zeros(D, np.float64)
    cprime = 1.0
    for i in range(L):
        Wv = wv[i].reshape(D, H * K).astype(np.float64)
        Wo = wo[i].reshape(H * K, D).astype(np.float64)
        cwi = cw[i].reshape(D).astype(np.float64)
        wocw = Wo @ cwi
        usum += Wv @ wocw
        cprime += float(bv[i].reshape(H * K).astype(np.float64) @ wocw)
        cprime += float(bo[i].astype(np.float64) @ cwi)
    cbsum = cb.astype(np.float64).sum(axis=0)
    return usum.astype(np.float32), float(np.float32(cprime)), cbsum.astype(np.float32)


def _ensure_trace_hook_importable():
    # bass_utils unconditionally imports antenv.axon_hooks when the
    # BASS_TRACE env var is set; some images lack that module. A None
    # hook makes bass_utils skip tracing gracefully.
    try:
        import antenv.axon_hooks  # noqa: F401
    except ImportError:
        import sys
        import types

        mod = types.ModuleType("antenv.axon_hooks")
        mod.get_axon_ntff_profile_hook = lambda: None
        mod.set_axon_ntff_profile_hook = lambda hook: None
        sys.modules["antenv.axon_hooks"] = mod


def kernel(x, wq, bq, wk, bk, wv, bv, wo, bo, cw, cb):
    from concourse.bass_utils import run_bass_kernel_spmd

    _ensure_trace_hook_importable()

    x = np.ascontiguousarray(np.asarray(x, dtype=np.float32))
    usum, cprime, cbsum = _precompute(
        np.asarray(wv), np.asarray(bv), np.asarray(wo), np.asarray(bo),
        np.asarray(cw), np.asarray(cb),
    )
    zero_cb = not np.any(cbsum)

    key = (cprime, zero_cb)
    if key not in _cache:
        _cache[key] = _build_program(cprime, zero_cb)
    nc = _cache[key]

    import ml_dtypes

    urow = np.concatenate([usum, np.float32(cprime).reshape(1)]).astype(
        ml_dtypes.bfloat16
    )
    u2 = np.ascontiguousarray(np.broadcast_to(urow.reshape(1, D + 1), (P, D + 1)))
    cb2 = cbsum.reshape(1, D)
    in_maps = [
        {"x": x[c * B_LOC : (c + 1) * B_LOC], "u": u2, "cb": cb2}
        for c in range(N_CORES)
    ]
    res = run_bass_kernel_spmd(nc, in_maps, list(range(N_CORES)))
    return np.concatenate([res.results[c]["out"] for c in range(N_CORES)], axis=0)


# revision 41
# speedup vs baseline: 1.0799x; 1.0799x over previous
"""Trainium2 Bass kernel for nn_AttentionCrossLayer.

Math: in the reference, softmax over a length-1 axis is exactly 1.0, so
attn == v and q/k/wq/wk are dead code. With x0 the (never-mutated) input,
each layer's gate xw_i = out_i @ cw_i is a fixed linear function of x0:
    xw_i = x0 @ u_i + c_i,   u_i = Wv_i @ (Wo_i @ cw_i),
                             c_i = (bv_i @ Wo_i + bo_i) @ cw_i
and the layer recurrence x += x0 * xw_i + cb_i telescopes to
    out[b, d] = x0[b, d] * (x0[b, :] @ usum + cprime) + cbsum[d]
with usum = sum_i u_i  [D], cprime = 1 + sum_i c_i, cbsum = sum_i cb_i [D].

The tiny weight contractions happen host-side in float64. On device,
per core: 32 row-tiles of [128, 1024], moved as eight 4-tile 2MB DMA
groups. Per tile:
  pass 1: fused multiply + row-reduce (scalar_tensor_tensor with
    accum_out) -> per-row gate t; cprime rides in a constant column
    appended to x/u so the reduce emits the finished gate.
  pass 2: in-place x <- x * t (+ cbsum on the general path).

Schedule (from per-queue trace A/B over nine variants):
  - All data rides the two HWDGE rings (sync + scalar), which together
    sustain the ~420 GB/s HBM rate. Each ring's stores are queued FIFO
    BEHIND its loads, so every load byte drains before any store byte:
    the load phase always runs at full rate and the store backlog
    drains opportunistically, overlapping the fixed ~8us epilogue
    (barrier + the NRT postamble's 249 serial semaphore resets). This
    removes the ring round-robin race between load and store rings
    that made earlier schedules bimodal (+-10us).
  - 2MB group DMAs keep each ring below its ~8-deep blocking window
    (an HWDGE ring BLOCKS the issuing engine when it backs up — that
    starved pass 2 and with it the store stream in one variant) and
    amortize the per-DMA ring-serialization gap that capped the ramp.
  - SWDGE (gpsimd) is never used at all: SWDGE loads concurrent with
    any other ring degrade the pool to ~300-345 GB/s, and Pool rejects
    the TensorScalarPtr compute ops at codegen.
  - Compute: pass 1 on Vector (STT + accum), pass 2 on Scalar
    (ACTIVATE with per-partition scale AP), pipelined one tile apart
    at ~1.25us/tile each. Per-tile pass-1 semaphores + per-group
    pass-2 counters order the cross-engine chain.
  - u arrives host-replicated to [128, D+1] bf16 (262KB, cprime column
    baked in): a single contiguous streaming load (DMA replication
    descriptors are HBM-latency-bound and poisoned the pool for ~15us;
    an fp32 PE broadcast costs ~5us in LDWEIGHTS+drain).
  - No final store waits: the NRT postamble + DMA quiesce guarantee
    completion; trailing stores drain under the epilogue. Verified
    bit-identical outputs across repeated runs.

Sharding: data-parallel over batch across 8 cores, weights replicated,
no cross-device comms.
"""

import numpy as np

L, B, D, H, K = 3, 32768, 1024, 8, 64
N_CORES = 8
B_LOC = B // N_CORES  # 4096 rows per core
P = 128
N_TILES = B_LOC // P  # 32
GSZ = 4  # tiles per DMA group
N_G = N_TILES // GSZ  # 8 groups
DP = D + 32  # slot stride 4224B = 128B aligned; col D holds the 1.0 constant

_cache = {}


def _build_program(cprime: float, zero_cb: bool):
    import concourse.bass as bass
    from concourse import mybir

    F32 = mybir.dt.float32
    BF16 = mybir.dt.bfloat16
    MUL = mybir.AluOpType.mult
    ADD = mybir.AluOpType.add
    BYP = mybir.AluOpType.bypass

    nc = bass.Bass()
    x = nc.declare_dram_parameter("x", [B_LOC, D], F32, isOutput=False)
    u = nc.declare_dram_parameter("u", [P, D + 1], BF16, isOutput=False)
    cb = nc.declare_dram_parameter("cb", [1, D], F32, isOutput=False)
    out = nc.declare_dram_parameter("out", [B_LOC, D], F32, isOutput=True)

    cb_bcast = bass.AP(tensor=cb.ap().tensor, offset=0, ap=[[0, P], [1, D]])

    # Eight uniform 4-tile load groups. HARD CONSTRAINT: an HWDGE ring
    # goes INVALID beyond ~9-10 queued DMAs (half the output silently
    # never stored at 11+), so each ring carries at most 9 DMAs
    # (u + 4 loads + 4 stores on sync; 4 + 4 on scalar). Graduated
    # (smaller-first) group sizes would start the compute chain ~12us
    # earlier but push the ring count past that limit.
    sizes = [4, 4, 4, 4, 4, 4, 4, 4]
    assert sum(sizes) == N_TILES
    lg_bounds = []
    pos = 0
    for s in sizes:
        lg_bounds.append((pos, pos + s))
        pos += s
    tile_lg = {}
    for gi, (a, b) in enumerate(lg_bounds):
        for t in range(a, b):
            tile_lg[t] = gi
    sy_lgroups = [g for g in range(len(sizes)) if g % 2 == 0]
    sc_lgroups = [g for g in range(len(sizes)) if g % 2 == 1]

    sy_groups = [0, 2, 4, 6]  # sync ring store groups (4 tiles each)
    sc_groups = [1, 3, 5, 7]  # scalar ring store groups

    # Compute split: pass 1 on Vector, pass 2 on Scalar (GpSimd/Pool
    # rejects TensorScalarPtr at codegen, so it can't help with either
    # pass). Each runs ~1.25us/tile, pipelined 1 tile apart.
    p2_sc = list(range(N_TILES))

    with (
        nc.sbuf_tensor([P, D + 1], BF16) as ub,  # [:, :D]=usum, [:, D]=cprime
        nc.sbuf_tensor([P, D], F32) as cbb,
        nc.sbuf_tensor([P, N_TILES, DP], F32) as xt,  # [:, i, D] = 1.0
        nc.sbuf_tensor([P, 4, D + 1], F32) as oscr,  # throwaway STT main out
        nc.sbuf_tensor([P, N_TILES, 1], F32) as tsc,
        nc.semaphore("ubb") as ubb,  # u block landed
        nc.semaphore("cbs") as cbs,  # cb broadcast landed (general path)
        nc.Block(no_gpsimd_drain=True) as block,
    ):
        lgs = [nc.alloc_semaphore(f"lg{g}") for g in range(len(sizes))]
        p1s = [nc.alloc_semaphore(f"p1_{i}") for i in range(N_TILES)]  # pass1
        cpg = [nc.alloc_semaphore(f"cp{g}") for g in range(N_G)]  # pass2 count
        stg = [nc.alloc_semaphore(f"sg{g}") for g in range(N_G)]  # store done

        def load_group(eng, g):
            a, b = lg_bounds[g]
            eng.dma_start(
                out=xt[:, a:b, 0:D],
                in_=x[a * P : b * P, :],
            ).then_inc(lgs[g], 16)

        def store_group(eng, g):
            eng.wait_ge(cpg[g], GSZ)
            eng.dma_start(
                out=out[g * GSZ * P : (g + 1) * GSZ * P, :],
                in_=xt[:, g * GSZ : (g + 1) * GSZ, 0:D],
            ).then_inc(stg[g], 16)

        def pass1(engine_ns, i, base=0):
            # oscr = x' * u' ; t_i = sum_free = x.usum + cprime
            # (oscr slots are disjoint per engine: vector 0-1, gpsimd 2-3)
            return engine_ns.scalar_tensor_tensor(
                out=oscr[:, base + (i % 2), :],
                in0=xt[:, i, 0 : D + 1],
                scalar=1.0,
                in1=ub[:, :],
                op0=MUL,
                op1=MUL,
                accum_out=tsc[:, i, :],
            ).then_inc(p1s[i], 1)

        @block.sync
        def _(sync):
            # 262KB u block first: lands by ~9.5us, gating only pass 1.
            sync.dma_start(out=ub[:, :], in_=u.ap()).then_inc(ubb, 16)
            for g in sy_lgroups:
                load_group(sync, g)
            for g in sy_groups:
                store_group(sync, g)

        @block.scalar
        def _(scalar):
            if not zero_cb:
                scalar.dma_start(out=cbb[:, :], in_=cb_bcast).then_inc(cbs, 16)
            for g in sc_lgroups:
                load_group(scalar, g)
            if zero_cb:
                for i in p2_sc:
                    scalar.wait_ge(p1s[i], 1)
                    nc.scalar.mul(
                        out=xt[:, i, 0:D],
                        in_=xt[:, i, 0:D],
                        mul=tsc[:, i, :],
                    ).then_inc(cpg[i // GSZ], 1)
                    # a group's pass 2 completes with its 4th mul; issue
                    # that store right away (ring is past its loads)
                    if i % (2 * GSZ) == 2 * GSZ - 1:
                        store_group(scalar, i // GSZ)
            else:
                for g in sc_groups:
                    store_group(scalar, g)

        @block.vector
        def _(vector):
            nc.vector.memset(xt[:, :, D : D + 1], 1.0)
            vector.wait_ge(ubb, 16)
            if not zero_cb:
                vector.wait_ge(cbs, 16)
                for i in range(N_TILES):
                    vector.wait_ge(lgs[tile_lg[i]], 16)
                    pass1(nc.vector, i)
                    vector.wait_ge(p1s[i], 1)  # accum writeback retired
                    nc.vector.scalar_tensor_tensor(
                        out=xt[:, i, 0:D],
                        in0=xt[:, i, 0:D],
                        scalar=tsc[:, i, :],
                        in1=cbb[:, :],
                        op0=MUL,
                        op1=ADD,
                    ).then_inc(cpg[i // GSZ], 1)
            else:
                for i in range(N_TILES):
                    vector.wait_ge(lgs[tile_lg[i]], 16)
                    pass1(nc.vector, i)

    return nc


def _precompute(wv, bv, wo, bo, cw, cb):
    """Host-side f64 contraction of the small per-layer weights."""
    usum = np.zeros(D, np.float64)
    cprime = 1.0
    for i in range(L):
        Wv = wv[i].reshape(D, H * K).astype(np.float64)
        Wo = wo[i].reshape(H * K, D).astype(np.float64)
        cwi = cw[i].reshape(D).astype(np.float64)
        wocw = Wo @ cwi
        usum += Wv @ wocw
        cprime += float(bv[i].reshape(H * K).astype(np.float64) @ wocw)
        cprime += float(bo[i].astype(np.float64) @ cwi)
    cbsum = cb.astype(np.float64).sum(axis=0)
    return usum.astype(np.float32), float(np.float32(cprime)), cbsum.astype(np.float32)


def _ensure_trace_hook_importable():
    # bass_utils unconditionally imports antenv.axon_hooks when the
    # BASS_TRACE env var is set; some images lack that module. A None
    # hook makes bass_utils skip tracing gracefully.
    try:
        import antenv.axon_hooks  # noqa: F401
    except ImportError:
        import sys
        import types

        mod = types.ModuleType("antenv.axon_hooks")
        mod.get_axon_ntff_profile_hook = lambda: None
        mod.set_axon_ntff_profile_hook = lambda hook: None
        sys.modules["antenv.axon_hooks"] = mod


def kernel(x, wq, bq, wk, bk, wv, bv, wo, bo, cw, cb):
    import ml_dtypes

    from concourse.bass_utils import run_bass_kernel_spmd

    _ensure_trace_hook_importable()

    x = np.ascontiguousarray(np.asarray(x, dtype=np.float32))
    usum, cprime, cbsum = _precompute(
        np.asarray(wv), np.asarray(bv), np.asarray(wo), np.asarray(bo),
        np.asarray(cw), np.asarray(cb),
    )
    zero_cb = not np.any(cbsum)

    key = (cprime, zero_cb)
    if key not in _cache:
        _cache[key] = _build_program(cprime, zero_cb)
    nc = _cache[key]

    urow = np.concatenate([usum, np.float32(cprime).reshape(1)]).astype(
        ml_dtypes.bfloat16
    )
    u2 = np.ascontiguousarray(np.broadcast_to(urow.reshape(1, D + 1), (P, D + 1)))
    cb2 = cbsum.reshape(1, D)
    in_maps = [
        {"x": x[c * B_LOC : (c + 1) * B_LOC], "u": u2, "cb": cb2}
        for c in range(N_CORES)
    ]
    res = run_bass_kernel_spmd(nc, in_maps, list(range(N_CORES)))
    return np.concatenate([res.results[c]["out"] for c in range(N_CORES)], axis=0)


# revision 52
# speedup vs baseline: 1.1352x; 1.0512x over previous
"""Trainium2 Bass kernel for nn_AttentionCrossLayer.

Math: in the reference, softmax over a length-1 axis is exactly 1.0, so
attn == v and q/k/wq/wk are dead code. With x0 the (never-mutated) input,
each layer's gate xw_i = out_i @ cw_i is a fixed linear function of x0:
    xw_i = x0 @ u_i + c_i,   u_i = Wv_i @ (Wo_i @ cw_i),
                             c_i = (bv_i @ Wo_i + bo_i) @ cw_i
and the layer recurrence x += x0 * xw_i + cb_i telescopes to
    out[b, d] = x0[b, d] * (x0[b, :] @ usum + cprime) + cbsum[d]
with usum = sum_i u_i  [D], cprime = 1 + sum_i c_i, cbsum = sum_i cb_i [D].

The tiny weight contractions happen host-side in float64. On device,
per core: 32 row-tiles of [128, 1024], moved as eight 4-tile 2MB DMA
groups. Per tile:
  pass 1: fused multiply + row-reduce (scalar_tensor_tensor with
    accum_out) -> per-row gate t; cprime rides in a constant column
    appended to x/u so the reduce emits the finished gate.
  pass 2: in-place x <- x * t (+ cbsum on the general path).

Schedule (from per-queue trace A/B over nine variants):
  - All data rides the two HWDGE rings (sync + scalar), which together
    sustain the ~420 GB/s HBM rate. Each ring's stores are queued FIFO
    BEHIND its loads, so every load byte drains before any store byte:
    the load phase always runs at full rate and the store backlog
    drains opportunistically, overlapping the fixed ~8us epilogue
    (barrier + the NRT postamble's 249 serial semaphore resets). This
    removes the ring round-robin race between load and store rings
    that made earlier schedules bimodal (+-10us).
  - 2MB group DMAs keep each ring below its ~8-deep blocking window
    (an HWDGE ring BLOCKS the issuing engine when it backs up — that
    starved pass 2 and with it the store stream in one variant) and
    amortize the per-DMA ring-serialization gap that capped the ramp.
  - SWDGE (gpsimd) is never used at all: SWDGE loads concurrent with
    any other ring degrade the pool to ~300-345 GB/s, and Pool rejects
    the TensorScalarPtr compute ops at codegen.
  - Compute: pass 1 on Vector (STT + accum), pass 2 on Scalar
    (ACTIVATE with per-partition scale AP), pipelined one tile apart
    at ~1.25us/tile each. Per-tile pass-1 semaphores + per-group
    pass-2 counters order the cross-engine chain.
  - u arrives host-replicated to [128, D+1] bf16 (262KB, cprime column
    baked in): a single contiguous streaming load (DMA replication
    descriptors are HBM-latency-bound and poisoned the pool for ~15us;
    an fp32 PE broadcast costs ~5us in LDWEIGHTS+drain).
  - No final store waits: the NRT postamble + DMA quiesce guarantee
    completion; trailing stores drain under the epilogue. Verified
    bit-identical outputs across repeated runs.

Sharding: data-parallel over batch across 8 cores, weights replicated,
no cross-device comms.
"""

import numpy as np

L, B, D, H, K = 3, 32768, 1024, 8, 64
N_CORES = 8
B_LOC = B // N_CORES  # 4096 rows per core
P = 128
N_TILES = B_LOC // P  # 32
GSZ = 4  # tiles per DMA group
N_G = N_TILES // GSZ  # 8 groups
DP = D + 32  # slot stride 4224B = 128B aligned; col D holds the 1.0 constant

_cache = {}


def _build_program(cprime: float, zero_cb: bool):
    import concourse.bass as bass
    from concourse import mybir

    F32 = mybir.dt.float32
    BF16 = mybir.dt.bfloat16
    MUL = mybir.AluOpType.mult
    ADD = mybir.AluOpType.add
    BYP = mybir.AluOpType.bypass

    nc = bass.Bass()
    x = nc.declare_dram_parameter("x", [B_LOC, D], F32, isOutput=False)
    u = nc.declare_dram_parameter("u", [P, D + 1], BF16, isOutput=False)
    cb = nc.declare_dram_parameter("cb", [1, D], F32, isOutput=False)
    out = nc.declare_dram_parameter("out", [B_LOC, D], F32, isOutput=True)

    cb_bcast = bass.AP(tensor=cb.ap().tensor, offset=0, ap=[[0, P], [1, D]])

    # Eight uniform 4-tile load groups. Schedule variations that look
    # faster on paper are land-mined (all HW-verified): >9-10 DMAs on
    # one HWDGE ring makes the queue invalid (half the output silently
    # never stored); 8-tile 4MB stores scramble the row mapping; and a
    # front-tapered 9-DMA/9-DMA split still dropped 1/4 of the output.
    # This exact 9/8-DMA shape is the measured-correct configuration.
    sizes = [4, 4, 4, 4, 4, 4, 4, 4]
    assert sum(sizes) == N_TILES
    lg_bounds = []
    pos = 0
    for s in sizes:
        lg_bounds.append((pos, pos + s))
        pos += s
    tile_lg = {}
    for gi, (a, b) in enumerate(lg_bounds):
        for t in range(a, b):
            tile_lg[t] = gi
    sy_lgroups = [g for g in range(len(sizes)) if g % 2 == 0]
    sc_lgroups = [g for g in range(len(sizes)) if g % 2 == 1]

    sy_groups = [0, 2, 4, 6]  # sync ring store groups (4 tiles each)
    sc_groups = [1, 3, 5, 7]  # scalar ring store groups

    # Compute split: pass 1 on Vector, pass 2 on Scalar (GpSimd/Pool
    # rejects TensorScalarPtr at codegen, so it can't help with either
    # pass). Each runs ~1.25us/tile, pipelined 1 tile apart.
    p2_sc = list(range(N_TILES))

    with (
        nc.sbuf_tensor([P, D + 1], BF16) as ub,  # [:, :D]=usum, [:, D]=cprime
        nc.sbuf_tensor([P, D], F32) as cbb,
        nc.sbuf_tensor([P, N_TILES, DP], F32) as xt,  # [:, i, D] = 1.0
        nc.sbuf_tensor([P, 4, D + 1], F32) as oscr,  # throwaway STT main out
        nc.sbuf_tensor([P, N_TILES, 1], F32) as tsc,
        nc.semaphore("ubb") as ubb,  # u block landed
        nc.semaphore("cbs") as cbs,  # cb broadcast landed (general path)
        nc.Block(no_gpsimd_drain=True) as block,
    ):
        lgs = [nc.alloc_semaphore(f"lg{g}") for g in range(len(sizes))]
        p1s = [nc.alloc_semaphore(f"p1_{i}") for i in range(N_TILES)]  # pass1
        cpg = [nc.alloc_semaphore(f"cp{g}") for g in range(N_G)]  # pass2 count
        stg = [nc.alloc_semaphore(f"sg{g}") for g in range(N_G)]  # store done

        def load_group(eng, g):
            a, b = lg_bounds[g]
            eng.dma_start(
                out=xt[:, a:b, 0:D],
                in_=x[a * P : b * P, :],
            ).then_inc(lgs[g], 16)

        def store_group(eng, g):
            eng.wait_ge(cpg[g], GSZ)
            eng.dma_start(
                out=out[g * GSZ * P : (g + 1) * GSZ * P, :],
                in_=xt[:, g * GSZ : (g + 1) * GSZ, 0:D],
            ).then_inc(stg[g], 16)

        def pass1(engine_ns, i, base=0):
            # oscr = x' * u' ; t_i = sum_free = x.usum + cprime
            # (oscr slots are disjoint per engine: vector 0-1, gpsimd 2-3)
            return engine_ns.scalar_tensor_tensor(
                out=oscr[:, base + (i % 2), :],
                in0=xt[:, i, 0 : D + 1],
                scalar=1.0,
                in1=ub[:, :],
                op0=MUL,
                op1=MUL,
                accum_out=tsc[:, i, :],
            ).then_inc(p1s[i], 1)

        @block.sync
        def _(sync):
            # 262KB u block first: lands by ~9.5us, gating only pass 1.
            sync.dma_start(out=ub[:, :], in_=u.ap()).then_inc(ubb, 16)
            for g in sy_lgroups:
                load_group(sync, g)
            for g in sy_groups:
                store_group(sync, g)

        @block.scalar
        def _(scalar):
            if not zero_cb:
                scalar.dma_start(out=cbb[:, :], in_=cb_bcast).then_inc(cbs, 16)
            for g in sc_lgroups:
                load_group(scalar, g)
            if zero_cb:
                for i in p2_sc:
                    scalar.wait_ge(p1s[i], 1)
                    nc.scalar.mul(
                        out=xt[:, i, 0:D],
                        in_=xt[:, i, 0:D],
                        mul=tsc[:, i, :],
                    ).then_inc(cpg[i // GSZ], 1)
                    # a group's pass 2 completes with its 4th mul; issue
                    # that store right away (ring is past its loads)
                    if i % (2 * GSZ) == 2 * GSZ - 1:
                        store_group(scalar, i // GSZ)
            else:
                for g in sc_groups:
                    store_group(scalar, g)

        @block.vector
        def _(vector):
            nc.vector.memset(xt[:, :, D : D + 1], 1.0)
            vector.wait_ge(ubb, 16)
            if not zero_cb:
                vector.wait_ge(cbs, 16)
                for i in range(N_TILES):
                    vector.wait_ge(lgs[tile_lg[i]], 16)
                    pass1(nc.vector, i)
                    vector.wait_ge(p1s[i], 1)  # accum writeback retired
                    nc.vector.scalar_tensor_tensor(
                        out=xt[:, i, 0:D],
                        in0=xt[:, i, 0:D],
                        scalar=tsc[:, i, :],
                        in1=cbb[:, :],
                        op0=MUL,
                        op1=ADD,
                    ).then_inc(cpg[i // GSZ], 1)
            else:
                for i in range(N_TILES):
                    vector.wait_ge(lgs[tile_lg[i]], 16)
                    pass1(nc.vector, i)

    return nc


def _precompute(wv, bv, wo, bo, cw, cb):
    """Host-side f64 contraction of the small per-layer weights."""
    usum = np.zeros(D, np.float64)
    cprime = 1.0
    for i in range(L):
        Wv = wv[i].reshape(D, H * K).astype(np.float64)
        Wo = wo[i].reshape(H * K, D).astype(np.float64)
        cwi = cw[i].reshape(D).astype(np.float64)
        wocw = Wo @ cwi
        usum += Wv @ wocw
        cprime += float(bv[i].reshape(H * K).astype(np.float64) @ wocw)
        cprime += float(bo[i].astype(np.float64) @ cwi)
    cbsum = cb.astype(np.float64).sum(axis=0)
    return usum.astype(np.float32), float(np.float32(cprime)), cbsum.astype(np.float32)


def _ensure_trace_hook_importable():
    # bass_utils unconditionally imports antenv.axon_hooks when the
    # BASS_TRACE env var is set; some images lack that module. A None
    # hook makes bass_utils skip tracing gracefully.
    try:
        import antenv.axon_hooks  # noqa: F401
    except ImportError:
        import sys
        import types

        mod = types.ModuleType("antenv.axon_hooks")
        mod.get_axon_ntff_profile_hook = lambda: None
        mod.set_axon_ntff_profile_hook = lambda hook: None
        sys.modules["antenv.axon_hooks"] = mod


def kernel(x, wq, bq, wk, bk, wv, bv, wo, bo, cw, cb):
    import ml_dtypes

    from concourse.bass_utils import run_bass_kernel_spmd

    _ensure_trace_hook_importable()

    x = np.ascontiguousarray(np.asarray(x, dtype=np.float32))
    usum, cprime, cbsum = _precompute(
        np.asarray(wv), np.asarray(bv), np.asarray(wo), np.asarray(bo),
        np.asarray(cw), np.asarray(cb),
    )
    zero_cb = not np.any(cbsum)

    key = (cprime, zero_cb)
    if key not in _cache:
        _cache[key] = _build_program(cprime, zero_cb)
    nc = _cache[key]

    urow = np.concatenate([usum, np.float32(cprime).reshape(1)]).astype(
        ml_dtypes.bfloat16
    )
    u2 = np.ascontiguousarray(np.broadcast_to(urow.reshape(1, D + 1), (P, D + 1)))
    cb2 = cbsum.reshape(1, D)
    in_maps = [
        {"x": x[c * B_LOC : (c + 1) * B_LOC], "u": u2, "cb": cb2}
        for c in range(N_CORES)
    ]
    res = run_bass_kernel_spmd(nc, in_maps, list(range(N_CORES)))
    return np.concatenate([res.results[c]["out"] for c in range(N_CORES)], axis=0)
